# revision 25
# baseline (speedup 1.0000x reference)
"""Trainium2 Bass kernel for nn_MoELayer (top-2 MoE, E=8 experts).

Strategy (tensor-parallel over the FFN dim, 8 NeuronCores):
  - Host computes the (tiny) gate matmul + top-2 + softmax and groups the
    2N=8192 (token, expert) pairs by expert.
  - Every core processes ALL 8192 pairs, but only a 512-wide slice of the
    FFN dimension F: core c uses W1[:, :, c*512:(c+1)*512] and
    W2[:, c*512:(c+1)*512, :], producing a PARTIAL down-projection.
    Host sums the 8 partials, applies gate weights, and scatter-adds into
    the output. This is perfectly load-balanced by construction: the
    per-expert routing imbalance (max count 1071 vs mean 1024 for the
    graded input) costs nothing, unlike expert-parallel capacity padding.
  - Per core: 2 * 8192 * 1024 * 512 MACs = 524288 PE-streaming cycles
    (218.5us at 2.4GHz) -- the bf16 tensor-engine floor.

Within a core, pairs are processed in chunks of <=512 tokens (PSUM bank
limit), grouped by expert so each chunk uses one expert's weight tiles
(all 8 experts' F-slices stay SBUF-resident: 2 x 64KB/partition).

  stage 1:  actT[f, c] = silu( sum_d W1s[d, f] * tokT[d, c] )   f in [0,512)
  stage 2:  partT[d, c] = sum_f actT[f, c] * W2s[f, d]

Emission is software-pipelined (stage-1 of chunk j+1 before stage-2 of
chunk j) so stage-2 never waits on the silu latency of its own chunk's
last f-tile.

DMA (single sync-engine HW DGE queue, transfers run in emission order):
  - weights/tokens stream just-in-time: w1[e0] in 4 slices, tok chunks and
    w2/w1 per expert interleaved ahead of their consumers.
  - outputs are staged per-chunk ([P, KD, cn] bf16) and their descriptors
    are interleaved into the queue 3 expert-groups behind the inputs so
    they never head-of-line-block an input the PE is about to need.
  - the last chunk's output goes out per-dm-tile so the end-of-kernel
    exposed transfer is ~100KB, not ~1MB.
A burst of dependency-free garbage matmuls is emitted first: it runs
during the input-DMA wait and warms the PE HAM clock-gate (else the
first ~3.4us of real matmuls run at 1.2GHz instead of 2.4GHz).
"""

import math
import sys

sys.path.insert(0, "/opt/trn_rl_repo")

import ml_dtypes
import numpy as np

B, T, D, F, E = 2, 2048, 1024, 4096, 8
N = B * T
P = 128
KD = D // P  # 8
FS = F // E  # 512 F-slice per core
KFS = FS // P  # 4
CMAX = 512

bf16 = ml_dtypes.bfloat16

_nc_cache: dict[tuple, object] = {}
LAST_RESULTS = None  # BassKernelResults from the most recent run (for test.py)
TRACE = False


def _plan(counts) -> list[tuple[int, int]]:
    """Chunk plan: list of (expert, cn). Near-equal chunks of <=512 per
    expert; expert 0's first chunk is capped at ~384 so the very first
    matmul's token DMA is small (fast pipeline start)."""
    plan = []
    for e in range(E):
        n = int(counts[e])
        if n == 0:
            continue
        sizes = []
        if e == 0 and n > CMAX:
            sizes.append(384)
            n -= 384
        tail = 0
        if e == E - 1 and n > CMAX:
            # small final chunk -> short exposed copy+DMA at kernel end
            tail = 256
            n -= 256
        k = max(1, math.ceil(n / CMAX))
        base, extra = divmod(n, k)
        sizes += [base + 1] * extra + [base] * (k - extra)
        if tail:
            sizes.append(tail)
        plan += [(e, cn) for cn in sizes if cn > 0]
    return plan


def _build(plan: tuple[tuple[int, int], ...]):
    import concourse.mybir as mybir
    import concourse.tile as tile
    from concourse import bacc

    dt = mybir.dt

    nc = bacc.Bacc(None, target_bir_lowering=False)

    nchunk = len(plan)
    tokts = [
        nc.dram_tensor(f"tokt{j}", [P, KD, cn], dt.bfloat16, kind="ExternalInput")
        for j, (e, cn) in enumerate(plan)
    ]
    # w1[p, e*KFS+fj, dk, fi] = W1slice[e][dk*P + p, fj*128 + fi]
    w1 = nc.dram_tensor("w1", [P, E * KFS, KD, P], dt.bfloat16, kind="ExternalInput")
    # w2[p, e*KD+dm, fk, di] = W2slice[e][fk*P + p, dm*128 + di]
    w2 = nc.dram_tensor("w2", [P, E * KD, KFS, P], dt.bfloat16, kind="ExternalInput")
    outs = [
        nc.dram_tensor(f"out{j}", [P, KD, cn], dt.bfloat16, kind="ExternalOutput")
        for j, (e, cn) in enumerate(plan)
    ]

    # chunk index ranges per expert group (for DMA interleaving)
    grp = [[] for _ in range(E)]
    for j, (e, cn) in enumerate(plan):
        grp[e].append(j)

    with tile.TileContext(nc) as tc:
        with (
            tc.tile_pool(name="const", bufs=1) as cpool,
            tc.tile_pool(name="tok", bufs=3) as tpool,
            tc.tile_pool(name="act", bufs=2) as apool,
            tc.tile_pool(name="stg", bufs=3) as spool,
            tc.tile_pool(name="ps1", bufs=3, space="PSUM") as ps1pool,
            tc.tile_pool(name="ps2", bufs=3, space="PSUM") as ps2pool,
            tc.tile_pool(name="warm", bufs=1, space="PSUM") as wpool,
        ):
            w1_sb = cpool.tile([P, E * KFS, KD, P], dt.bfloat16, tag="w1")
            w2_sb = cpool.tile([P, E * KD, KFS, P], dt.bfloat16, tag="w2")

            # ---- PE warm-up: matmuls on a zeroed scratch tile, no DMA
            # deps. ~3.5us of PE activity starting as soon as the engine
            # is free -> HAM un-throttles to 2.4GHz before the first real
            # matmul (and finishes before its input DMAs land). ----
            warm_sb = cpool.tile([P, 192], dt.bfloat16, tag="warm_sb")
            nc.vector.memset(warm_sb[:], 0)
            wps = wpool.tile([P, 64], dt.float32, tag="warm")
            for _ in range(115):
                nc.tensor.matmul(
                    wps[:], warm_sb[:, :128], warm_sb[:, 128:192],
                    start=True, stop=True, skip_group_check=True,
                )

            tok_sbs = {}
            act_sbs = {}
            stg_sbs = {}

            def load_tok(j):
                e, cn = plan[j]
                t = tpool.tile([P, KD, cn], dt.bfloat16, tag="tok",
                               name=f"tok_sb{j}")
                tok_sbs[j] = t
                nc.sync.dma_start(t[:], tokts[j][:])

            # ---- input DMA stream (emission order == transfer order) ----
            # expert 0: w1 slice fj=0 first, first tok chunk, rest of w1
            nc.sync.dma_start(w1_sb[:, 0], w1[:, 0])
            load_tok(grp[0][0])
            for fj in range(1, KFS):
                nc.sync.dma_start(w1_sb[:, fj], w1[:, fj])
            mid = grp[0][1:]
            if mid:
                load_tok(mid[0])
            # w2[e0] in two halves around the next tok chunk: stage 2 of
            # chunk 0 starts right after stage 1 of chunk 1 and must not
            # wait on the full-expert w2 transfer (nor on every e0 tok)
            nc.sync.dma_start(w2_sb[:, 0 : KD // 2], w2[:, 0 : KD // 2])
            if len(mid) > 1:
                load_tok(mid[1])
            nc.sync.dma_start(w2_sb[:, KD // 2 : KD], w2[:, KD // 2 : KD])
            for j in mid[2:]:
                load_tok(j)
            for e in range(1, 3):
                nc.sync.dma_start(
                    w1_sb[:, e * KFS : (e + 1) * KFS], w1[:, e * KFS : (e + 1) * KFS]
                )
                for j in grp[e]:
                    load_tok(j)
                nc.sync.dma_start(
                    w2_sb[:, e * KD : (e + 1) * KD], w2[:, e * KD : (e + 1) * KD]
                )

            out_emitted = 0

            def emit_outs(upto):
                nonlocal out_emitted
                while out_emitted < upto:
                    j = out_emitted
                    if j == nchunk - 1:
                        # two halves: the first transfers while dm 4-7
                        # still compute, halving the exposed end-of-kernel
                        # transfer (3+ pieces measured worse: per-descriptor
                        # issue serialization at the queue tail)
                        for lo, hi in ((0, KD // 2), (KD // 2, KD)):
                            nc.sync.dma_start(
                                outs[j][:, lo:hi, :], stg_sbs[j][:, lo:hi, :]
                            )
                    else:
                        nc.sync.dma_start(outs[j][:], stg_sbs[j][:])
                    del stg_sbs[j]
                    out_emitted += 1

            def emit_inputs(e):
                nc.sync.dma_start(
                    w1_sb[:, e * KFS : (e + 1) * KFS], w1[:, e * KFS : (e + 1) * KFS]
                )
                for j in grp[e]:
                    load_tok(j)
                nc.sync.dma_start(
                    w2_sb[:, e * KD : (e + 1) * KD], w2[:, e * KD : (e + 1) * KD]
                )

            # remaining input groups + lagging output descriptors are
            # emitted inside the compute loop below (so the queue stays
            # input-ahead / output-behind).

            def stage1(j):
                e, cn = plan[j]
                tok_sb = tok_sbs.pop(j)
                a = apool.tile([P, KFS, cn], dt.bfloat16, tag="act",
                               name=f"act_sb{j}")
                act_sbs[j] = a
                for fj in range(KFS):
                    ps1 = ps1pool.tile([P, cn], dt.float32, tag="ps1")
                    for dk in range(KD):
                        nc.tensor.matmul(
                            ps1[:],
                            w1_sb[:, e * KFS + fj, dk],
                            tok_sb[:, dk, :],
                            start=(dk == 0),
                            stop=(dk == KD - 1),
                        )
                    nc.scalar.activation(
                        a[:, fj, :], ps1[:], mybir.ActivationFunctionType.Silu
                    )

            def stage2(j):
                e, cn = plan[j]
                a = act_sbs.pop(j)
                stg = spool.tile([P, KD, cn], dt.bfloat16, tag="stg",
                                 name=f"stg_sb{j}")
                stg_sbs[j] = stg
                for dm in range(KD):
                    ps2 = ps2pool.tile([P, cn], dt.float32, tag="ps2")
                    for fk in range(KFS):
                        nc.tensor.matmul(
                            ps2[:],
                            w2_sb[:, e * KD + dm, fk],
                            a[:, fk, :],
                            start=(fk == 0),
                            stop=(fk == KFS - 1),
                        )
                    nc.vector.tensor_copy(stg[:, dm, :], ps2[:])

            # ---- software-pipelined chunk loop ----
            # order: s1(0), s1(1), s2(0), s1(2), s2(1), ... s2(last)
            # inputs for expert group e land 3 groups ahead of use;
            # output descriptors trail ~1 group behind production.
            next_in_grp = 3
            stage1(0)
            for j in range(1, nchunk):
                e_j = plan[j][0]
                while next_in_grp <= min(e_j + 2, E - 1):
                    emit_inputs(next_in_grp)
                    next_in_grp += 1
                stage1(j)
                stage2(j - 1)
                emit_outs(j - 1)
            while next_in_grp < E:
                emit_inputs(next_in_grp)
                next_in_grp += 1
            stage2(nchunk - 1)
            emit_outs(nchunk)

    nc.compile()
    return nc


def _get_nc(plan):
    key = tuple(plan)
    if key not in _nc_cache:
        _nc_cache[key] = _build(key)
    return _nc_cache[key]


def kernel(**inputs) -> np.ndarray:
    global LAST_RESULTS
    x = np.asarray(inputs["x"], dtype=np.float32)
    Wg = np.asarray(inputs["Wg"], dtype=np.float32)
    W1 = np.asarray(inputs["W1"], dtype=np.float32)
    W2 = np.asarray(inputs["W2"], dtype=np.float32)

    h = np.ascontiguousarray(x.reshape(N, D))

    # ---- host gate: top-2 + softmax (0.05% of total FLOPs) ----
    logits = h @ Wg.T  # [N, E] f32
    idx2 = np.argpartition(-logits, 1, axis=1)[:, :2]
    lsel = np.take_along_axis(logits, idx2, axis=1)
    first = lsel[:, 0] >= lsel[:, 1]
    i0 = np.where(first, idx2[:, 0], idx2[:, 1])
    i1 = np.where(first, idx2[:, 1], idx2[:, 0])
    l0 = np.where(first, lsel[:, 0], lsel[:, 1])
    l1 = np.where(first, lsel[:, 1], lsel[:, 0])
    e1 = np.exp((l1 - l0).astype(np.float32))
    w0 = (1.0 / (1.0 + e1)).astype(np.float32)
    w1g = (e1 / (1.0 + e1)).astype(np.float32)

    token_ids = np.concatenate([np.arange(N), np.arange(N)])
    expert_ids = np.concatenate([i0, i1])
    gate_w = np.concatenate([w0, w1g])

    counts = np.bincount(expert_ids, minlength=E)
    plan = _plan(counts)

    hb = h.astype(bf16)
    W1b = W1.astype(bf16)
    W2b = W2.astype(bf16)

    # pair order: grouped by expert (matches the chunk plan)
    order_pairs = np.concatenate(
        [np.flatnonzero(expert_ids == e) for e in range(E)]
    )
    tids = token_ids[order_pairs]
    gws = gate_w[order_pairs]

    # tokens [2N,D] -> [D,2N] -> [KD,P,2N] -> [P,KD,2N], then chunked
    tokT = np.ascontiguousarray(
        hb[tids].T.reshape(KD, P, 2 * N).transpose(1, 0, 2)
    )
    tok_chunks = {}
    c0 = 0
    for j, (e, cn) in enumerate(plan):
        tok_chunks[f"tokt{j}"] = np.ascontiguousarray(tokT[:, :, c0 : c0 + cn])
        c0 += cn
    assert c0 == 2 * N

    in_maps = []
    for c in range(E):
        # core c's F-slice of every expert's weights
        w1p = np.stack(
            [
                # [D, FS] -> [KD, P, KFS, 128] -> [P, KFS, KD, 128]
                W1b[e][:, c * FS : (c + 1) * FS]
                .reshape(KD, P, KFS, P)
                .transpose(1, 2, 0, 3)
                for e in range(E)
            ],
            axis=1,
        ).reshape(P, E * KFS, KD, P)
        w2p = np.stack(
            [
                # [FS, D] -> [KFS, P, KD, 128] -> [P, KD, KFS, 128]
                W2b[e][c * FS : (c + 1) * FS, :]
                .reshape(KFS, P, KD, P)
                .transpose(1, 2, 0, 3)
                for e in range(E)
            ],
            axis=1,
        ).reshape(P, E * KD, KFS, P)
        m = {"w1": np.ascontiguousarray(w1p), "w2": np.ascontiguousarray(w2p)}
        m.update(tok_chunks)
        in_maps.append(m)

    nc = _get_nc(plan)
    from concourse.bass_utils import run_bass_kernel_spmd

    LAST_RESULTS = run_bass_kernel_spmd(
        nc, in_maps, core_ids=list(range(E)), trace=TRACE
    )

    # ---- combine: sum the 8 partial outputs, gate-weight, pair-reduce ----
    Ot = np.zeros((P, KD, 2 * N), dtype=np.float32)
    for c in range(E):
        c0 = 0
        for j, (e, cn) in enumerate(plan):
            Ot[:, :, c0 : c0 + cn] += np.asarray(
                LAST_RESULTS.results[c][f"out{j}"], dtype=np.float32
            )
            c0 += cn
    # Ot[p, dm, col] = partial_out[dm*128+p, col] -> [2N, D]
    contrib = Ot.transpose(2, 1, 0).reshape(2 * N, D)
    contrib *= gws[:, None]
    srt = np.argsort(tids, kind="stable")
    cs = contrib[srt]
    y = cs[0::2] + cs[1::2]
    return y.reshape(B, T, D)
